# revision 4
# baseline (speedup 1.0000x reference)
# Trainium2 Bass kernel for nn_BinLinearEval:
#   out[b, o] = (round(x @ W.T + bias) * sign >= 0) ? 1.0 : 0.0
#
# Math folding (exact because bias is integer-valued and sign in {-1,+1}):
#   out = 1  iff  sign*(dot + bias) >= -0.5
#       = 1  iff  dot' >= thr_o      where dot' = x @ (sign.T*W).T  (W' still
#         ternary, exact in fp16) and thr_o = -sign_o*bias_o - 0.5.
# So the device computes one matmul (accumulated over a hi/lo fp16 split of x
# for fp32-level accuracy) and a single per-partition is_ge threshold.
#
# Sharding: data-parallel over batch, 8192 rows per core. x is pre-transposed
# on the host to [feature, batch] layout so the contract dim lands on SBUF
# partitions; output is produced as [out, batch] per core and re-assembled /
# transposed on the host.

import os
from contextlib import ExitStack

import numpy as np
import ml_dtypes

BATCH, IN_F, OUT_F = 65536, 1024, 256
N_CORES = 8
B_CORE = BATCH // N_CORES  # 8192
P = 128
KC = IN_F // P             # 8 k-chunks
OC = OUT_F // P            # 2 out-channel chunks
GROUP = 1024               # batch columns per DMA group
BT = 512                   # matmul moving free dim
N_GROUPS = B_CORE // GROUP
IO_BUFS = 3
WARMUP_MMS = 120

_CACHE = {}


def _build():
    """Build (and cache) the Bass module. Returns the compiled nc."""
    if "nc" in _CACHE:
        return _CACHE["nc"]

    import concourse.bacc as bacc
    import concourse.mybir as mybir
    import concourse.tile as tile

    nc = bacc.Bacc(
        "TRN2",
        target_bir_lowering=False,
        debug=False,
        num_devices=N_CORES,
    )

    f16 = mybir.dt.float16
    f32 = mybir.dt.float32
    bf16 = mybir.dt.bfloat16

    xhi_d = nc.dram_tensor("xhi", [P, KC, B_CORE], f16, kind="ExternalInput").ap()
    xlo_d = nc.dram_tensor("xlo", [P, KC, B_CORE], f16, kind="ExternalInput").ap()
    wt_d = nc.dram_tensor("wt", [P, KC, OUT_F], f16, kind="ExternalInput").ap()
    thr_d = nc.dram_tensor("thr", [P, OC], f32, kind="ExternalInput").ap()
    out_d = nc.dram_tensor("out", [OC, P, B_CORE], bf16, kind="ExternalOutput").ap()

    with tile.TileContext(nc) as tc, ExitStack() as ctx:
        const = ctx.enter_context(tc.tile_pool(name="const", bufs=1))
        io = ctx.enter_context(tc.tile_pool(name="io", bufs=IO_BUFS))
        outp = ctx.enter_context(tc.tile_pool(name="outp", bufs=4))
        psum = ctx.enter_context(tc.tile_pool(name="psum", bufs=4, space="PSUM"))

        wt_sb = const.tile([P, KC, OUT_F], f16)
        nc.sync.dma_start(out=wt_sb, in_=wt_d)
        thr_sb = const.tile([P, OC], f32)
        nc.sync.dma_start(out=thr_sb, in_=thr_d)

        # PE pre-warm: tiny independent matmuls on scratch data keep the
        # HAM activity monitor busy while the first input DMAs land, so the
        # real matmul stream starts at 2.4 GHz instead of 1.2 GHz.
        warm_in = const.tile([P, 64], f16)
        nc.vector.memset(warm_in, 0.0)
        warm_ps = ctx.enter_context(
            tc.tile_pool(name="warm_ps", bufs=2, space="PSUM")
        )
        for _ in range(WARMUP_MMS):
            wps = warm_ps.tile([64, 64], f32, name="wps")
            nc.tensor.matmul(wps, warm_in, warm_in, start=True, stop=True)

        for g in range(N_GROUPS):
            g0 = g * GROUP
            xh = []
            xl = []
            for k in range(KC):
                h = io.tile([P, GROUP], f16, name=f"xh{k}")
                l = io.tile([P, GROUP], f16, name=f"xl{k}")
                nc.sync.dma_start(out=h, in_=xhi_d[:, k, g0 : g0 + GROUP])
                nc.sync.dma_start(out=l, in_=xlo_d[:, k, g0 : g0 + GROUP])
                xh.append(h)
                xl.append(l)
            for bt in range(GROUP // BT):
                b0 = bt * BT
                for oc in range(OC):
                    ps = psum.tile([P, BT], f32, name="ps")
                    for k in range(KC):
                        w_ap = wt_sb[:, k, oc * P : (oc + 1) * P]
                        nc.tensor.matmul(
                            ps,
                            w_ap,
                            xh[k][:, b0 : b0 + BT],
                            start=(k == 0),
                            stop=False,
                        )
                        nc.tensor.matmul(
                            ps,
                            w_ap,
                            xl[k][:, b0 : b0 + BT],
                            start=False,
                            stop=(k == KC - 1),
                        )
                    ob = outp.tile([P, BT], bf16, name="ob")
                    nc.vector.tensor_scalar(
                        ob,
                        ps,
                        thr_sb[:, oc : oc + 1],
                        None,
                        mybir.AluOpType.is_ge,
                    )
                    nc.sync.dma_start(
                        out=out_d[oc, :, g0 + b0 : g0 + b0 + BT], in_=ob
                    )

    nc.compile()
    _CACHE["nc"] = nc
    return nc


def _prep_inputs(x, weight, bias, sign):
    """Host-side prep: fold sign into weights, build thresholds, split x into
    fp16 hi/lo, transpose to [feature, batch] per-core tiles."""
    x = np.asarray(x, dtype=np.float32)
    weight = np.asarray(weight, dtype=np.float32)
    bias = np.asarray(bias, dtype=np.float32)
    sign = np.asarray(sign, dtype=np.float32).reshape(1, OUT_F)

    wp = sign.T * weight                      # [OUT_F, IN_F], ternary
    thr = (-sign[0] * bias - np.float32(0.5)).astype(np.float32)  # [OUT_F]

    wt = np.ascontiguousarray(
        wp.T.reshape(KC, P, OUT_F).transpose(1, 0, 2)
    ).astype(np.float16)                      # [P, KC, OUT_F]
    thr2 = np.ascontiguousarray(thr.reshape(OC, P).T)  # [P, OC]

    xhi = x.astype(np.float16)
    xlo = (x - xhi.astype(np.float32)).astype(np.float16)

    in_maps = []
    for c in range(N_CORES):
        sl = slice(c * B_CORE, (c + 1) * B_CORE)
        hi = np.ascontiguousarray(
            xhi[sl].reshape(B_CORE, KC, P).transpose(2, 1, 0)
        )                                      # [P, KC, B_CORE]
        lo = np.ascontiguousarray(
            xlo[sl].reshape(B_CORE, KC, P).transpose(2, 1, 0)
        )
        in_maps.append({"xhi": hi, "xlo": lo, "wt": wt, "thr": thr2})
    return in_maps


def _assemble(results):
    """[core][OC, P, B_CORE] bf16 -> [BATCH, OUT_F] fp32"""
    full = np.concatenate(
        [r["out"].reshape(OUT_F, B_CORE) for r in results], axis=1
    )  # [OUT_F, BATCH]
    return np.ascontiguousarray(full.T).astype(np.float32)


def run(x, weight, bias, sign, trace=False):
    """Run the kernel; returns (output, BassKernelResults)."""
    from concourse.bass_utils import run_bass_kernel_spmd

    nc = _build()
    in_maps = _prep_inputs(x, weight, bias, sign)
    res = run_bass_kernel_spmd(
        nc,
        in_maps,
        core_ids=list(range(N_CORES)),
        trace=trace,
    )
    return _assemble(res.results), res


def kernel(x, weight, bias, sign):
    out, _ = run(x, weight, bias, sign, trace=False)
    return out


# revision 7
# speedup vs baseline: 1.1688x; 1.1688x over previous
# Trainium2 Bass kernel for nn_BinLinearEval:
#   out[b, o] = (round(x @ W.T + bias) * sign >= 0) ? 1.0 : 0.0
#
# Math folding (exact because bias is integer-valued and sign in {-1,+1}):
#   out = 1  iff  sign*(dot + bias) >= -0.5
#       = 1  iff  dot' >= thr_o      where dot' = x @ (sign.T*W).T  (W' still
#         ternary, exact in fp16) and thr_o = -sign_o*bias_o - 0.5.
# So the device computes one matmul (accumulated over a hi/lo fp16 split of x
# for fp32-level accuracy) and a single per-partition is_ge threshold.
#
# Sharding: data-parallel over batch, 8192 rows per core. x is pre-transposed
# on the host to [feature, batch] layout so the contract dim lands on SBUF
# partitions; output is produced as [out, batch] per core and re-assembled /
# transposed on the host.

import os
from contextlib import ExitStack

import numpy as np
import ml_dtypes

BATCH, IN_F, OUT_F = 65536, 1024, 256
N_CORES = 8
B_CORE = BATCH // N_CORES  # 8192
P = 128
KC = IN_F // P             # 8 k-chunks
OC = OUT_F // P            # 2 out-channel chunks
BT = 512                   # matmul moving free dim
# Batch columns per DMA group: small groups first so the PE starts after
# ~1 MB of input DMA instead of ~4.5 MB, larger groups later for DMA
# efficiency. Must sum to B_CORE, each a multiple of BT.
GROUPS = [512, 512, 1024, 1024, 1024, 1024, 1024, 1024, 1024]
assert sum(GROUPS) == B_CORE
IO_BUFS = 3

_CACHE = {}


def _build():
    """Build (and cache) the Bass module. Returns the compiled nc."""
    if "nc" in _CACHE:
        return _CACHE["nc"]

    import concourse.bacc as bacc
    import concourse.mybir as mybir
    import concourse.tile as tile

    nc = bacc.Bacc(
        "TRN2",
        target_bir_lowering=False,
        debug=False,
        num_devices=N_CORES,
    )

    f16 = mybir.dt.float16
    f32 = mybir.dt.float32
    bf16 = mybir.dt.bfloat16

    xhi_d = nc.dram_tensor("xhi", [P, KC, B_CORE], f16, kind="ExternalInput").ap()
    xlo_d = nc.dram_tensor("xlo", [P, KC, B_CORE], f16, kind="ExternalInput").ap()
    wt_d = nc.dram_tensor("wt", [P, KC, OUT_F], f16, kind="ExternalInput").ap()
    thr_d = nc.dram_tensor("thr", [P, OC], f32, kind="ExternalInput").ap()
    out_d = nc.dram_tensor("out", [OC, P, B_CORE], bf16, kind="ExternalOutput").ap()

    with tile.TileContext(nc) as tc, ExitStack() as ctx:
        const = ctx.enter_context(tc.tile_pool(name="const", bufs=1))
        io = ctx.enter_context(tc.tile_pool(name="io", bufs=IO_BUFS))
        outp = ctx.enter_context(tc.tile_pool(name="outp", bufs=4))
        psum = ctx.enter_context(tc.tile_pool(name="psum", bufs=4, space="PSUM"))

        wt_sb = const.tile([P, KC, OUT_F], f16)
        nc.sync.dma_start(out=wt_sb, in_=wt_d)
        thr_sb = const.tile([P, OC], f32)
        nc.sync.dma_start(out=thr_sb, in_=thr_d)

        g0 = 0
        for g, group in enumerate(GROUPS):
            xhi_sb = io.tile([P, KC, max(GROUPS)], f16, name="xhi_sb")[
                :, :, :group
            ]
            xlo_sb = io.tile([P, KC, max(GROUPS)], f16, name="xlo_sb")[
                :, :, :group
            ]
            nc.sync.dma_start(out=xhi_sb, in_=xhi_d[:, :, g0 : g0 + group])
            nc.sync.dma_start(out=xlo_sb, in_=xlo_d[:, :, g0 : g0 + group])
            for bt in range(group // BT):
                b0 = bt * BT
                for oc in range(OC):
                    ps = psum.tile([P, BT], f32, name="ps")
                    for k in range(KC):
                        w_ap = wt_sb[:, k, oc * P : (oc + 1) * P]
                        nc.tensor.matmul(
                            ps,
                            w_ap,
                            xhi_sb[:, k, b0 : b0 + BT],
                            start=(k == 0),
                            stop=False,
                        )
                        nc.tensor.matmul(
                            ps,
                            w_ap,
                            xlo_sb[:, k, b0 : b0 + BT],
                            start=False,
                            stop=(k == KC - 1),
                        )
                    ob = outp.tile([P, BT], bf16, name="ob")
                    nc.vector.tensor_scalar(
                        ob,
                        ps,
                        thr_sb[:, oc : oc + 1],
                        None,
                        mybir.AluOpType.is_ge,
                    )
                    nc.sync.dma_start(
                        out=out_d[oc, :, g0 + b0 : g0 + b0 + BT], in_=ob
                    )
            g0 += group

    nc.compile()
    _CACHE["nc"] = nc
    return nc


def _prep_inputs(x, weight, bias, sign):
    """Host-side prep: fold sign into weights, build thresholds, split x into
    fp16 hi/lo, transpose to [feature, batch] per-core tiles."""
    x = np.asarray(x, dtype=np.float32)
    weight = np.asarray(weight, dtype=np.float32)
    bias = np.asarray(bias, dtype=np.float32)
    sign = np.asarray(sign, dtype=np.float32).reshape(1, OUT_F)

    wp = sign.T * weight                      # [OUT_F, IN_F], ternary
    thr = (-sign[0] * bias - np.float32(0.5)).astype(np.float32)  # [OUT_F]

    wt = np.ascontiguousarray(
        wp.T.reshape(KC, P, OUT_F).transpose(1, 0, 2)
    ).astype(np.float16)                      # [P, KC, OUT_F]
    thr2 = np.ascontiguousarray(thr.reshape(OC, P).T)  # [P, OC]

    xhi = x.astype(np.float16)
    xlo = (x - xhi.astype(np.float32)).astype(np.float16)

    in_maps = []
    for c in range(N_CORES):
        sl = slice(c * B_CORE, (c + 1) * B_CORE)
        hi = np.ascontiguousarray(
            xhi[sl].reshape(B_CORE, KC, P).transpose(2, 1, 0)
        )                                      # [P, KC, B_CORE]
        lo = np.ascontiguousarray(
            xlo[sl].reshape(B_CORE, KC, P).transpose(2, 1, 0)
        )
        in_maps.append({"xhi": hi, "xlo": lo, "wt": wt, "thr": thr2})
    return in_maps


def _assemble(results):
    """[core][OC, P, B_CORE] bf16 -> [BATCH, OUT_F] fp32"""
    full = np.concatenate(
        [r["out"].reshape(OUT_F, B_CORE) for r in results], axis=1
    )  # [OUT_F, BATCH]
    return np.ascontiguousarray(full.T).astype(np.float32)


def run(x, weight, bias, sign, trace=False):
    """Run the kernel; returns (output, BassKernelResults)."""
    from concourse.bass_utils import run_bass_kernel_spmd

    nc = _build()
    in_maps = _prep_inputs(x, weight, bias, sign)
    res = run_bass_kernel_spmd(
        nc,
        in_maps,
        core_ids=list(range(N_CORES)),
        trace=trace,
    )
    return _assemble(res.results), res


def kernel(x, weight, bias, sign):
    out, _ = run(x, weight, bias, sign, trace=False)
    return out


# revision 9
# speedup vs baseline: 2.1506x; 1.8399x over previous
# Trainium2 Bass kernel for nn_BinLinearEval:
#   out[b, o] = (round(x @ W.T + bias) * sign >= 0) ? 1.0 : 0.0
#
# Math folding (exact because bias is integer-valued and sign in {-1,+1}):
#   out = 1  iff  sign*(dot + bias) >= -0.5
#       = 1  iff  dot' >= thr_o      where dot' = x @ (sign.T*W).T  (W' still
#         ternary, exact in fp16) and thr_o = -sign_o*bias_o - 0.5.
# So the device computes one matmul (accumulated over a hi/lo fp16 split of x
# for fp32-level accuracy) and a single per-partition is_ge threshold.
#
# Sharding: data-parallel over batch, 8192 rows per core. x is pre-transposed
# on the host to [feature, batch] layout so the contract dim lands on SBUF
# partitions; output is produced as [out, batch] per core and re-assembled /
# transposed on the host.

import os
from contextlib import ExitStack

import numpy as np
import ml_dtypes

BATCH, IN_F, OUT_F = 65536, 1024, 256
N_CORES = 8
B_CORE = BATCH // N_CORES  # 8192
P = 128
KC = IN_F // P             # 8 k-chunks
OC = OUT_F // P            # 2 out-channel chunks
BT = 512                   # matmul moving free dim
# Uniform small groups + deep buffering: DMA stays saturated and the PE
# never outruns the prefetch pipeline by more than the buffer depth.
GROUPS = [512] * (B_CORE // 512)
assert sum(GROUPS) == B_CORE
IO_BUFS = 6

_CACHE = {}


def _build():
    """Build (and cache) the Bass module. Returns the compiled nc."""
    if "nc" in _CACHE:
        return _CACHE["nc"]

    import concourse.bacc as bacc
    import concourse.mybir as mybir
    import concourse.tile as tile

    nc = bacc.Bacc(
        "TRN2",
        target_bir_lowering=False,
        debug=False,
        num_devices=N_CORES,
    )

    f16 = mybir.dt.float16
    f32 = mybir.dt.float32
    bf16 = mybir.dt.bfloat16

    xhi_d = nc.dram_tensor("xhi", [P, KC, B_CORE], f16, kind="ExternalInput").ap()
    xlo_d = nc.dram_tensor("xlo", [P, KC, B_CORE], f16, kind="ExternalInput").ap()
    wt_d = nc.dram_tensor("wt", [P, KC, OUT_F], f16, kind="ExternalInput").ap()
    thr_d = nc.dram_tensor("thr", [P, OC], f32, kind="ExternalInput").ap()
    out_d = nc.dram_tensor("out", [OC, P, B_CORE], bf16, kind="ExternalOutput").ap()

    with tile.TileContext(nc) as tc, ExitStack() as ctx:
        const = ctx.enter_context(tc.tile_pool(name="const", bufs=1))
        io = ctx.enter_context(tc.tile_pool(name="io", bufs=IO_BUFS))
        outp = ctx.enter_context(tc.tile_pool(name="outp", bufs=4))
        psum = ctx.enter_context(tc.tile_pool(name="psum", bufs=4, space="PSUM"))

        wt_sb = const.tile([P, KC, OUT_F], f16)
        nc.sync.dma_start(out=wt_sb, in_=wt_d)
        thr_sb = const.tile([P, OC], f32)
        nc.sync.dma_start(out=thr_sb, in_=thr_d)

        g0 = 0
        for g, group in enumerate(GROUPS):
            xhi_sb = io.tile([P, KC, max(GROUPS)], f16, name="xhi_sb")[
                :, :, :group
            ]
            xlo_sb = io.tile([P, KC, max(GROUPS)], f16, name="xlo_sb")[
                :, :, :group
            ]
            nc.sync.dma_start(out=xhi_sb, in_=xhi_d[:, :, g0 : g0 + group])
            nc.sync.dma_start(out=xlo_sb, in_=xlo_d[:, :, g0 : g0 + group])
            for bt in range(group // BT):
                b0 = bt * BT
                for oc in range(OC):
                    ps = psum.tile([P, BT], f32, name="ps")
                    # all-hi then all-lo: the first matmuls of the kernel
                    # only need the hi half of the first group in SBUF
                    for k in range(KC):
                        nc.tensor.matmul(
                            ps,
                            wt_sb[:, k, oc * P : (oc + 1) * P],
                            xhi_sb[:, k, b0 : b0 + BT],
                            start=(k == 0),
                            stop=False,
                        )
                    for k in range(KC):
                        nc.tensor.matmul(
                            ps,
                            wt_sb[:, k, oc * P : (oc + 1) * P],
                            xlo_sb[:, k, b0 : b0 + BT],
                            start=False,
                            stop=(k == KC - 1),
                        )
                    ob = outp.tile([P, BT], bf16, name="ob")
                    nc.vector.tensor_scalar(
                        ob,
                        ps,
                        thr_sb[:, oc : oc + 1],
                        None,
                        mybir.AluOpType.is_ge,
                    )
                    # out-DMAs ride the ACT HWDGE ring so they never block
                    # the input-DMA FIFO on the SP ring
                    nc.scalar.dma_start(
                        out=out_d[oc, :, g0 + b0 : g0 + b0 + BT], in_=ob
                    )
            g0 += group

    nc.compile()
    _CACHE["nc"] = nc
    return nc


def _prep_inputs(x, weight, bias, sign):
    """Host-side prep: fold sign into weights, build thresholds, split x into
    fp16 hi/lo, transpose to [feature, batch] per-core tiles."""
    x = np.asarray(x, dtype=np.float32)
    weight = np.asarray(weight, dtype=np.float32)
    bias = np.asarray(bias, dtype=np.float32)
    sign = np.asarray(sign, dtype=np.float32).reshape(1, OUT_F)

    wp = sign.T * weight                      # [OUT_F, IN_F], ternary
    thr = (-sign[0] * bias - np.float32(0.5)).astype(np.float32)  # [OUT_F]

    wt = np.ascontiguousarray(
        wp.T.reshape(KC, P, OUT_F).transpose(1, 0, 2)
    ).astype(np.float16)                      # [P, KC, OUT_F]
    thr2 = np.ascontiguousarray(thr.reshape(OC, P).T)  # [P, OC]

    xhi = x.astype(np.float16)
    xlo = (x - xhi.astype(np.float32)).astype(np.float16)

    in_maps = []
    for c in range(N_CORES):
        sl = slice(c * B_CORE, (c + 1) * B_CORE)
        hi = np.ascontiguousarray(
            xhi[sl].reshape(B_CORE, KC, P).transpose(2, 1, 0)
        )                                      # [P, KC, B_CORE]
        lo = np.ascontiguousarray(
            xlo[sl].reshape(B_CORE, KC, P).transpose(2, 1, 0)
        )
        in_maps.append({"xhi": hi, "xlo": lo, "wt": wt, "thr": thr2})
    return in_maps


def _assemble(results):
    """[core][OC, P, B_CORE] bf16 -> [BATCH, OUT_F] fp32"""
    full = np.concatenate(
        [r["out"].reshape(OUT_F, B_CORE) for r in results], axis=1
    )  # [OUT_F, BATCH]
    return np.ascontiguousarray(full.T).astype(np.float32)


def run(x, weight, bias, sign, trace=False):
    """Run the kernel; returns (output, BassKernelResults)."""
    from concourse.bass_utils import run_bass_kernel_spmd

    nc = _build()
    in_maps = _prep_inputs(x, weight, bias, sign)
    res = run_bass_kernel_spmd(
        nc,
        in_maps,
        core_ids=list(range(N_CORES)),
        trace=trace,
    )
    return _assemble(res.results), res


def kernel(x, weight, bias, sign):
    out, _ = run(x, weight, bias, sign, trace=False)
    return out
